# revision 30
# baseline (speedup 1.0000x reference)
"""DecoderRNN (image-caption LSTM decoder) Trainium2 kernel.

Data-parallel over batch across 8 NeuronCores (16 batch rows per core).
All LSTM/embedding/fc parameters are replicated; each core computes its
batch shard end-to-end (mean-pool + feature projection, embedding gather,
32-step LSTM, vocab projection) and writes its [16, 32, 10000] output slice.

Self-contained: hardcodes all shapes; host-side numpy only reshapes /
transposes / concatenates weights and computes gather indices.
"""

import os
import numpy as np

import concourse.bass as bass
import concourse.mybir as mybir
import concourse.tile as tile
from concourse import bacc
from concourse.bass_utils import run_bass_kernel_spmd
from concourse.masks import make_identity

F32 = mybir.dt.float32
BF16 = mybir.dt.bfloat16
I32 = mybir.dt.int32

EMBED, HIDDEN, VOCAB, FEAT = 256, 512, 10000, 1024
B, NREG, T = 128, 49, 32  # T = CAPLEN - 1
NCORES = 8
BL = B // NCORES          # 16 batch rows per core
ROWS = BL * NREG          # 784 feature rows per core
NRT = (ROWS + 127) // 128  # 7 feature row tiles
G4 = 4 * HIDDEN           # 2048 gate width, order [g(c), i, f, o]
NT = T * BL               # 512 (t, b) rows per core

_CACHE = {}


def _build():
    if "nc" in _CACHE:
        return _CACHE["nc"]

    nc = bacc.Bacc("TRN2", target_bir_lowering=False, debug=False)

    # ---------------- DRAM I/O ----------------
    feat_d = nc.dram_tensor("feat", [ROWS, FEAT], BF16, kind="ExternalInput")
    sel_d = nc.dram_tensor("sel", [NRT, 128, BL], BF16, kind="ExternalInput")
    idx_d = nc.dram_tensor("idx", [4, 128, 1], I32, kind="ExternalInput")
    emb_d = nc.dram_tensor("emb", [VOCAB, EMBED], BF16, kind="ExternalInput")
    wft_d = nc.dram_tensor("wft", [8, 128, EMBED], BF16, kind="ExternalInput")
    bft_d = nc.dram_tensor("bft", [1, EMBED], BF16, kind="ExternalInput")
    wst_d = nc.dram_tensor("wst", [4, 128, G4], BF16, kind="ExternalInput")
    ball_d = nc.dram_tensor("ball", [1, G4], BF16, kind="ExternalInput")
    u_d = nc.dram_tensor("u", [4, 128, G4], BF16, kind="ExternalInput")
    wfc_d = nc.dram_tensor("wfc", [4, 128, VOCAB], BF16, kind="ExternalInput")
    bfc_d = nc.dram_tensor("bfc", [1, VOCAB], BF16, kind="ExternalInput")
    out_d = nc.dram_tensor("out", [BL, T, VOCAB], F32, kind="ExternalOutput")

    with tile.TileContext(nc) as tc:
        with (
            tc.tile_pool(name="const", bufs=1) as cp,
            tc.tile_pool(name="state", bufs=1) as sp,
        ):
            ident = cp.tile([128, 128], F32, tag="ident")
            make_identity(nc, ident[:])
            identb = cp.tile([128, 128], BF16, tag="identb")
            nc.vector.tensor_copy(identb[:], ident[:])
            ones = cp.tile([1, 128], BF16, tag="ones")
            nc.gpsimd.memset(ones[:], 1.0)
            # 32x32 identity tiled down all partitions: sels[p, q] = (p%32 == q)
            sels = cp.tile([128, 32], BF16, tag="sels")
            for a in range(4):
                nc.vector.tensor_copy(sels[32 * a:32 * (a + 1), :],
                                      ident[0:32, 0:32])

            # persistent state
            U = [sp.tile([128, G4], BF16, name=f"u{k}", tag=f"u{k}")
                 for k in range(4)]
            for k in range(4):
                nc.sync.dma_start(U[k][:], u_d[k])
            # per-step gate pre-acts; 8 steps per 2048-wide chunk, step t at
            # partitions [16*(t%8), 16*(t%8)+16) of chunk t//8
            Z = sp.tile([128, 4 * G4], BF16, tag="Z")
            hsT = [sp.tile([128, 16 * (T + 1)], BF16, name=f"hsT{k}", tag=f"hsT{k}")
                   for k in range(4)]                 # h^T history, slot 0 = h0 = 0
            for k in range(4):
                nc.gpsimd.memset(hsT[k][:], 0.0)
            c_sb = sp.tile([BL, HIDDEN], F32, tag="c_sb")  # LSTM cell state
            nc.gpsimd.memset(c_sb[:], 0.0)
            ET = [sp.tile([128, NT], BF16, name=f"ET{m}", tag=f"ET{m}")
                  for m in range(2)]
            Frep = [sp.tile([128, 128], BF16, name=f"Frep{m}", tag=f"Frep{m}")
                    for m in range(2)]

            # ---------------- prologue ----------------
            with tc.tile_pool(name="wpool", bufs=1) as wp:
                ball_s = wp.tile([1, G4], BF16, tag="ball")
                nc.sync.dma_start(ball_s[:], ball_d[:])
                bft_s = wp.tile([1, EMBED], BF16, tag="bft")
                nc.sync.dma_start(bft_s[:], bft_d[:])
                # feature tiles + selection matmul: fm = sel^T @ feat  -> [BL, FEAT]
                with tc.tile_pool(name="pA", bufs=2, space="PSUM") as pA:
                    fm_ps = pA.tile([BL, FEAT], F32, tag="fm_ps", bufs=1)
                    for kt in range(NRT):
                        ft = wp.tile([128, FEAT], BF16, name=f"ft{kt}", tag=f"ft{kt}")
                        r0 = kt * 128
                        nrows = min(128, ROWS - r0)
                        if nrows < 128:
                            nc.gpsimd.memset(ft[:], 0.0)
                        nc.sync.dma_start(ft[:nrows, :], feat_d[r0:r0 + nrows, :])
                        st = wp.tile([128, BL], BF16, name=f"st{kt}", tag=f"st{kt}")
                        nc.sync.dma_start(st[:], sel_d[kt])
                        for j in range(2):
                            nc.tensor.matmul(
                                fm_ps[:, j * 512:(j + 1) * 512], st[:],
                                ft[:, j * 512:(j + 1) * 512],
                                start=(kt == 0), stop=(kt == NRT - 1))
                    fm_sb = wp.tile([BL, FEAT], BF16, tag="fm_sb")
                    nc.scalar.copy(fm_sb[:], fm_ps[:])

                    # transpose fm -> fmT (8 x [128, BL])
                    fmT = [wp.tile([128, BL], BF16, name=f"fmT{kt}", tag=f"fmT{kt}")
                           for kt in range(8)]
                    for kt in range(8):
                        tp = pA.tile([128, BL], BF16, tag="trp")
                        nc.tensor.transpose(
                            tp[:], fm_sb[:, kt * 128:(kt + 1) * 128],
                            identb[:BL, :BL])
                        nc.vector.tensor_copy(fmT[kt][:], tp[:])

                    # feats_emb^T = wftT-chunks^T @ fmT + b_ft  -> fsb [2][128, BL]
                    for m in range(2):
                        fps = pA.tile([128, BL], F32, tag="fps", bufs=1)
                        for kt in range(8):
                            wt = wp.tile([128, EMBED], BF16, name="wtft",
                                         tag="wtft", bufs=2)
                            nc.sync.dma_start(wt[:], wft_d[kt])
                            nc.tensor.matmul(
                                fps[:], wt[:, m * 128:(m + 1) * 128], fmT[kt][:],
                                start=(kt == 0), stop=False)
                        nc.tensor.matmul(
                            fps[:], bft_s[0:1, m * 128:(m + 1) * 128],
                            ones[0:1, :BL], start=False, stop=True)
                        fsb = wp.tile([128, BL], BF16, name=f"fsb{m}", tag=f"fsb{m}")
                        nc.vector.tensor_copy(fsb[:], fps[:])
                        nc.vector.tensor_copy(
                            Frep[m][:].rearrange("p (s b) -> p s b", s=8),
                            fsb[:].unsqueeze(1).to_broadcast([128, 8, BL]))

                    # embedding gather -> E_nat [4][128, EMBED], rows (t, b)
                    Enat = [wp.tile([128, EMBED], BF16, name=f"en{rc}", tag=f"en{rc}")
                            for rc in range(4)]
                    for rc in range(4):
                        it = wp.tile([128, 1], I32, name=f"it{rc}", tag=f"it{rc}")
                        nc.sync.dma_start(it[:], idx_d[rc])
                        nc.gpsimd.indirect_dma_start(
                            out=Enat[rc][:], out_offset=None,
                            in_=emb_d[:],
                            in_offset=bass.IndirectOffsetOnAxis(ap=it[:, 0:1], axis=0))
                    # transpose E_nat -> ET [2][128, NT]
                    for rc in range(4):
                        for m in range(2):
                            tp2 = pA.tile([128, 128], BF16, tag="trp2")
                            nc.tensor.transpose(
                                tp2[:], Enat[rc][:, m * 128:(m + 1) * 128], identb[:])
                            nc.vector.tensor_copy(
                                ET[m][:, rc * 128:(rc + 1) * 128], tp2[:])

                # Z precompute: Z[(tsub,b), gate @ chunk mc] for t = 8*mc + tsub
                Wst = [wp.tile([128, G4], BF16, name=f"wst{k}", tag=f"wst{k}")
                       for k in range(4)]
                for k in range(4):
                    nc.sync.dma_start(Wst[k][:], wst_d[k])
                with tc.tile_pool(name="pZ", bufs=1, space="PSUM") as pZ:
                    for c in range(4):
                        zps = pZ.tile([128, G4], F32, tag="zps")
                        lhs = [ET[0][:, c * 128:(c + 1) * 128],
                               ET[1][:, c * 128:(c + 1) * 128],
                               Frep[0][:], Frep[1][:]]
                        for j in range(4):
                            sl = slice(j * 512, (j + 1) * 512)
                            for k in range(4):
                                nc.tensor.matmul(zps[:, sl], lhs[k], Wst[k][:, sl],
                                                 start=(k == 0), stop=False)
                            nc.tensor.matmul(zps[:, sl], ones[0:1, :],
                                             ball_s[0:1, sl], start=False, stop=True)
                        if c % 2 == 0:
                            nc.scalar.copy(Z[:, c * G4:(c + 1) * G4], zps[:])
                        else:
                            nc.vector.tensor_copy(Z[:, c * G4:(c + 1) * G4], zps[:])

            # W_fc^T fully resident (bf16, 10MB) + bias
            WFC = [sp.tile([128, VOCAB], BF16, name=f"wfcs{k}", tag=f"wfcs{k}")
                   for k in range(4)]
            for k in range(4):
                nc.sync.dma_start(WFC[k][:], wfc_d[k])
            bfc_s = sp.tile([1, VOCAB], BF16, tag="bfc_s")
            nc.sync.dma_start(bfc_s[:], bfc_d[:])

            # FC work queue: (mc, v0, w) — unit (mc, ...) becomes runnable
            # once LSTM step 8*mc+7 has written its h; interleave up to 3
            # units into each step's PE gap, drain the rest after the loop.
            vchunks = []
            v0 = 0
            while v0 < VOCAB:
                vchunks.append((v0, min(512, VOCAB - v0)))
                v0 += 512
            fc_queue = [(mc, v0, w) for mc in range(4) for (v0, w) in vchunks]
            fc_pos = 0

            # ---------------- LSTM loop + interleaved FC ----------------
            with (
                tc.tile_pool(name="lps", bufs=1, space="PSUM") as lps,
                tc.tile_pool(name="trpp", bufs=2, space="PSUM") as trpp,
                tc.tile_pool(name="fps2", bufs=2, space="PSUM") as fps2,
                tc.tile_pool(name="lsb", bufs=2) as lsb,
                tc.tile_pool(name="fsb2", bufs=4) as fsb2,
            ):
                def fc_unit(mc, v0, w):
                    hsl = slice((8 * mc + 1) * 16, (8 * mc + 9) * 16)
                    ps = fps2.tile([128, 512], F32, tag="fcp")
                    for k in range(4):
                        nc.tensor.matmul(
                            ps[:, :w], hsT[k][:, hsl], WFC[k][:, v0:v0 + w],
                            start=(k == 0), stop=False)
                    nc.tensor.matmul(
                        ps[:, :w], ones[0:1, :], bfc_s[0:1, v0:v0 + w],
                        start=False, stop=True)
                    stg = fsb2.tile([128, 512], F32, tag="fst")
                    if v0 % 1024 == 0:
                        nc.scalar.copy(stg[:, :w], ps[:, :w])
                    else:
                        nc.vector.tensor_copy(stg[:, :w], ps[:, :w])
                    nc.sync.dma_start(
                        out_d[:, 8 * mc:8 * mc + 8, v0:v0 + w]
                        .transpose([1, 0, 2]),
                        stg[:, :w])

                for t in range(T):
                    # select step-t rows of Z from its 32-aligned row pair:
                    # lhsT = I16 block picking lower/upper 16 of the group
                    zrow = 32 * ((t % 8) // 2)
                    zsel = (sels[zrow:zrow + 32, 0:16] if t % 2 == 0
                            else sels[zrow:zrow + 32, 16:32])
                    zoff = (t // 8) * G4
                    gps = lps.tile([BL, G4], F32, tag="gps")
                    for j in range(4):
                        sl = slice(j * 512, (j + 1) * 512)
                        nc.tensor.matmul(
                            gps[:, sl], zsel,
                            Z[zrow:zrow + 32, zoff + j * 512:zoff + (j + 1) * 512],
                            start=True, stop=False, tile_position=(zrow, 0))
                        for k in range(4):
                            nc.tensor.matmul(
                                gps[:, sl], hsT[k][:, t * 16:t * 16 + 16],
                                U[k][:, sl], start=False, stop=(k == 3))
                    # fill the elementwise-chain PE gap with ready FC units:
                    # these are independent of h_t, so they execute while the
                    # ACT/DVE chain below produces h_t
                    nfill = 0
                    while (fc_pos < len(fc_queue) and nfill < 3
                           and 8 * fc_queue[fc_pos][0] + 7 <= t - 1):
                        fc_unit(*fc_queue[fc_pos])
                        fc_pos += 1
                        nfill += 1
                    sact = lsb.tile([BL, G4], F32, tag="sact")
                    nc.scalar.activation(sact[:, 0:512], gps[:, 0:512],
                                         mybir.ActivationFunctionType.Tanh)
                    nc.scalar.activation(sact[:, 512:2048], gps[:, 512:2048],
                                         mybir.ActivationFunctionType.Sigmoid)
                    ig = lsb.tile([BL, 512], F32, tag="ig")
                    nc.vector.tensor_tensor(ig[:], sact[:, 512:1024],
                                            sact[:, 0:512], mybir.AluOpType.mult)
                    fc = lsb.tile([BL, 512], F32, tag="fc")
                    nc.vector.tensor_tensor(fc[:], sact[:, 1024:1536],
                                            c_sb[:], mybir.AluOpType.mult)
                    nc.vector.tensor_tensor(c_sb[:], ig[:], fc[:],
                                            mybir.AluOpType.add)
                    tch = lsb.tile([BL, 512], F32, tag="tch")
                    nc.scalar.activation(tch[:], c_sb[:],
                                         mybir.ActivationFunctionType.Tanh)
                    h = lsb.tile([BL, 512], F32, tag="h")
                    nc.vector.tensor_tensor(h[:], sact[:, 1536:2048],
                                            tch[:], mybir.AluOpType.mult)
                    tp3 = trpp.tile([128, 4 * BL], F32, tag="htr")
                    for k in range(4):
                        nc.tensor.transpose(
                            tp3[:, k * BL:(k + 1) * BL],
                            h[:, k * 128:(k + 1) * 128], ident[:BL, :BL])
                        nc.vector.tensor_copy(
                            hsT[k][:, (t + 1) * 16:(t + 2) * 16],
                            tp3[:, k * BL:(k + 1) * BL])
                # drain the remaining FC units
                while fc_pos < len(fc_queue):
                    fc_unit(*fc_queue[fc_pos])
                    fc_pos += 1

    nc.compile()
    _CACHE["nc"] = nc
    return nc


def _host_prep(inputs):
    """Build the 8 per-core input maps from full-size inputs."""
    f32 = lambda x: np.ascontiguousarray(np.asarray(x), dtype=np.float32)
    features = f32(inputs["features"])
    captions = np.asarray(inputs["captions"])
    emb = f32(inputs["emb"])
    w_ft = f32(inputs["W_ft"])

    # gate order [g(c), i, f, o]
    gates = ["c", "i", "f", "o"]
    w_stack = np.concatenate([f32(inputs[f"W_{g}"]) for g in gates], axis=1)
    u_all = np.concatenate([f32(inputs[f"U_{g}"]) for g in gates], axis=1)
    b_all = np.concatenate([f32(inputs[f"b_{g}"]) for g in gates])
    w_fc = f32(inputs["W_fc"])
    b_fc = f32(inputs["b_fc"])
    b_ft = f32(inputs["b_ft"])

    import ml_dtypes
    bf16 = ml_dtypes.bfloat16

    sel = np.zeros((NRT * 128, BL), np.float32)
    rows = np.arange(ROWS)
    sel[rows, rows // NREG] = 1.0 / NREG
    sel = sel.reshape(NRT, 128, BL)

    shared = {
        "sel": sel.astype(bf16),
        "emb": emb.astype(bf16),
        "wft": np.ascontiguousarray(w_ft.T).reshape(8, 128, EMBED).astype(bf16),
        "bft": b_ft.reshape(1, EMBED).astype(bf16),
        "wst": w_stack.reshape(4, 128, G4).astype(bf16),
        "ball": b_all.reshape(1, G4).astype(bf16),
        "u": u_all.reshape(4, 128, G4).astype(bf16),
        "wfc": np.ascontiguousarray(w_fc.T).reshape(4, 128, VOCAB).astype(bf16),
        "bfc": b_fc.reshape(1, VOCAB).astype(bf16),
    }
    in_maps = []
    for c in range(NCORES):
        bsl = slice(c * BL, (c + 1) * BL)
        idx = np.ascontiguousarray(
            captions[bsl, :T].astype(np.int64).T).astype(np.int32)  # [T, BL]
        m = dict(shared)
        m["feat"] = features[bsl].reshape(ROWS, FEAT).astype(bf16)
        m["idx"] = idx.reshape(4, 128, 1)
        in_maps.append(m)
    return in_maps


def _enable_ntff_hook():
    """Register the axon NTFF profile hook (missing antenv.axon_hooks shim)."""
    import sys
    import types
    try:
        from antenv.axon_hooks import get_axon_ntff_profile_hook  # noqa: F401
        return
    except ImportError:
        pass
    mod = types.ModuleType("antenv.axon_hooks")
    holder = [None]
    mod.set_axon_ntff_profile_hook = lambda h: holder.__setitem__(0, h)
    mod.get_axon_ntff_profile_hook = lambda: holder[0]
    sys.modules["antenv.axon_hooks"] = mod
    import antenv
    antenv.axon_hooks = mod
    from trn_agent_boot.trn_boot import _ntff_profile_via_ctypes
    mod.set_axon_ntff_profile_hook(
        _ntff_profile_via_ctypes("/opt/axon/libaxon_pjrt.so"))
    from concourse import bass_utils as bu
    bu.upload_artifacts = lambda tmpdir: f"local:{tmpdir}"


def kernel(**inputs) -> np.ndarray:
    nc = _build()
    in_maps = _host_prep(inputs)
    trace = bool(int(os.environ.get("DECODER_KERNEL_TRACE", "0")))
    kw = {}
    if trace:
        _enable_ntff_hook()
        tdir = os.environ.get("DECODER_KERNEL_TRACE_DIR")
        if tdir:
            os.makedirs(tdir, exist_ok=True)
            kw["tmpdir"] = tdir
    res = run_bass_kernel_spmd(nc, in_maps, list(range(NCORES)), trace=trace, **kw)
    _CACHE["exec_time_ns"] = res.exec_time_ns
    _CACHE["results_obj"] = res
    out = np.concatenate([res.results[c]["out"] for c in range(NCORES)], axis=0)
    return out


# revision 31
# speedup vs baseline: 1.0130x; 1.0130x over previous
"""DecoderRNN (image-caption LSTM decoder) Trainium2 kernel.

Data-parallel over batch across 8 NeuronCores (16 batch rows per core).
All LSTM/embedding/fc parameters are replicated; each core computes its
batch shard end-to-end (mean-pool + feature projection, embedding gather,
32-step LSTM, vocab projection) and writes its [16, 32, 10000] output slice.

Self-contained: hardcodes all shapes; host-side numpy only reshapes /
transposes / concatenates weights and computes gather indices.
"""

import os
import numpy as np

import concourse.bass as bass
import concourse.mybir as mybir
import concourse.tile as tile
from concourse import bacc
from concourse.bass_utils import run_bass_kernel_spmd
from concourse.masks import make_identity

F32 = mybir.dt.float32
BF16 = mybir.dt.bfloat16
I32 = mybir.dt.int32

EMBED, HIDDEN, VOCAB, FEAT = 256, 512, 10000, 1024
B, NREG, T = 128, 49, 32  # T = CAPLEN - 1
NCORES = 8
BL = B // NCORES          # 16 batch rows per core
ROWS = BL * NREG          # 784 feature rows per core
NRT = (ROWS + 127) // 128  # 7 feature row tiles
G4 = 4 * HIDDEN           # 2048 gate width, order [g(c), i, f, o]
NT = T * BL               # 512 (t, b) rows per core

_CACHE = {}


def _build():
    if "nc" in _CACHE:
        return _CACHE["nc"]

    nc = bacc.Bacc("TRN2", target_bir_lowering=False, debug=False)

    # ---------------- DRAM I/O ----------------
    feat_d = nc.dram_tensor("feat", [ROWS, FEAT], BF16, kind="ExternalInput")
    sel_d = nc.dram_tensor("sel", [NRT, 128, BL], BF16, kind="ExternalInput")
    idx_d = nc.dram_tensor("idx", [4, 128, 1], I32, kind="ExternalInput")
    emb_d = nc.dram_tensor("emb", [VOCAB, EMBED], BF16, kind="ExternalInput")
    wft_d = nc.dram_tensor("wft", [8, 128, EMBED], BF16, kind="ExternalInput")
    bft_d = nc.dram_tensor("bft", [1, EMBED], BF16, kind="ExternalInput")
    wst_d = nc.dram_tensor("wst", [4, 128, G4], BF16, kind="ExternalInput")
    ball_d = nc.dram_tensor("ball", [1, G4], BF16, kind="ExternalInput")
    u_d = nc.dram_tensor("u", [4, 128, G4], BF16, kind="ExternalInput")
    wfc_d = nc.dram_tensor("wfc", [4, 128, VOCAB], BF16, kind="ExternalInput")
    bfc_d = nc.dram_tensor("bfc", [1, VOCAB], BF16, kind="ExternalInput")
    out_d = nc.dram_tensor("out", [BL, T, VOCAB], F32, kind="ExternalOutput")

    with tile.TileContext(nc) as tc:
        with (
            tc.tile_pool(name="const", bufs=1) as cp,
            tc.tile_pool(name="state", bufs=1) as sp,
        ):
            ident = cp.tile([128, 128], F32, tag="ident")
            make_identity(nc, ident[:])
            identb = cp.tile([128, 128], BF16, tag="identb")
            nc.vector.tensor_copy(identb[:], ident[:])
            ones = cp.tile([1, 128], BF16, tag="ones")
            nc.gpsimd.memset(ones[:], 1.0)
            # 32x32 identity tiled down all partitions: sels[p, q] = (p%32 == q)
            sels = cp.tile([128, 32], BF16, tag="sels")
            for a in range(4):
                nc.vector.tensor_copy(sels[32 * a:32 * (a + 1), :],
                                      ident[0:32, 0:32])

            # persistent state
            U = [sp.tile([128, G4], BF16, name=f"u{k}", tag=f"u{k}")
                 for k in range(4)]
            for k in range(4):
                nc.sync.dma_start(U[k][:], u_d[k])
            # per-step gate pre-acts; 8 steps per 2048-wide chunk, step t at
            # partitions [16*(t%8), 16*(t%8)+16) of chunk t//8
            Z = sp.tile([128, 4 * G4], BF16, tag="Z")
            hsT = [sp.tile([128, 16 * (T + 1)], BF16, name=f"hsT{k}", tag=f"hsT{k}")
                   for k in range(4)]                 # h^T history, slot 0 = h0 = 0
            for k in range(4):
                nc.gpsimd.memset(hsT[k][:], 0.0)
            c_sb = sp.tile([BL, HIDDEN], F32, tag="c_sb")  # LSTM cell state
            nc.gpsimd.memset(c_sb[:], 0.0)
            ET = [sp.tile([128, NT], BF16, name=f"ET{m}", tag=f"ET{m}")
                  for m in range(2)]
            Frep = [sp.tile([128, 128], BF16, name=f"Frep{m}", tag=f"Frep{m}")
                    for m in range(2)]

            # ---------------- prologue ----------------
            with tc.tile_pool(name="wpool", bufs=1) as wp:
                ball_s = wp.tile([1, G4], BF16, tag="ball")
                nc.sync.dma_start(ball_s[:], ball_d[:])
                bft_s = wp.tile([1, EMBED], BF16, tag="bft")
                nc.sync.dma_start(bft_s[:], bft_d[:])
                # feature tiles + selection matmul: fm = sel^T @ feat  -> [BL, FEAT]
                with tc.tile_pool(name="pA", bufs=2, space="PSUM") as pA:
                    fm_ps = pA.tile([BL, FEAT], F32, tag="fm_ps", bufs=1)
                    for kt in range(NRT):
                        ft = wp.tile([128, FEAT], BF16, name=f"ft{kt}", tag=f"ft{kt}")
                        r0 = kt * 128
                        nrows = min(128, ROWS - r0)
                        if nrows < 128:
                            nc.gpsimd.memset(ft[:], 0.0)
                        nc.sync.dma_start(ft[:nrows, :], feat_d[r0:r0 + nrows, :])
                        st = wp.tile([128, BL], BF16, name=f"st{kt}", tag=f"st{kt}")
                        nc.sync.dma_start(st[:], sel_d[kt])
                        for j in range(2):
                            nc.tensor.matmul(
                                fm_ps[:, j * 512:(j + 1) * 512], st[:],
                                ft[:, j * 512:(j + 1) * 512],
                                start=(kt == 0), stop=(kt == NRT - 1))
                    fm_sb = wp.tile([BL, FEAT], BF16, tag="fm_sb")
                    nc.scalar.copy(fm_sb[:], fm_ps[:])

                    # transpose fm -> fmT (8 x [128, BL])
                    fmT = [wp.tile([128, BL], BF16, name=f"fmT{kt}", tag=f"fmT{kt}")
                           for kt in range(8)]
                    for kt in range(8):
                        tp = pA.tile([128, BL], BF16, tag="trp")
                        nc.tensor.transpose(
                            tp[:], fm_sb[:, kt * 128:(kt + 1) * 128],
                            identb[:BL, :BL])
                        nc.vector.tensor_copy(fmT[kt][:], tp[:])

                    # feats_emb^T = wftT-chunks^T @ fmT + b_ft  -> fsb [2][128, BL]
                    for m in range(2):
                        fps = pA.tile([128, BL], F32, tag="fps", bufs=1)
                        for kt in range(8):
                            wt = wp.tile([128, EMBED], BF16, name="wtft",
                                         tag="wtft", bufs=2)
                            nc.sync.dma_start(wt[:], wft_d[kt])
                            nc.tensor.matmul(
                                fps[:], wt[:, m * 128:(m + 1) * 128], fmT[kt][:],
                                start=(kt == 0), stop=False)
                        nc.tensor.matmul(
                            fps[:], bft_s[0:1, m * 128:(m + 1) * 128],
                            ones[0:1, :BL], start=False, stop=True)
                        fsb = wp.tile([128, BL], BF16, name=f"fsb{m}", tag=f"fsb{m}")
                        nc.vector.tensor_copy(fsb[:], fps[:])
                        nc.vector.tensor_copy(
                            Frep[m][:].rearrange("p (s b) -> p s b", s=8),
                            fsb[:].unsqueeze(1).to_broadcast([128, 8, BL]))

                    # embedding gather -> E_nat [4][128, EMBED], rows (t, b)
                    Enat = [wp.tile([128, EMBED], BF16, name=f"en{rc}", tag=f"en{rc}")
                            for rc in range(4)]
                    for rc in range(4):
                        it = wp.tile([128, 1], I32, name=f"it{rc}", tag=f"it{rc}")
                        nc.sync.dma_start(it[:], idx_d[rc])
                        nc.gpsimd.indirect_dma_start(
                            out=Enat[rc][:], out_offset=None,
                            in_=emb_d[:],
                            in_offset=bass.IndirectOffsetOnAxis(ap=it[:, 0:1], axis=0))
                    # transpose E_nat -> ET [2][128, NT]
                    for rc in range(4):
                        for m in range(2):
                            tp2 = pA.tile([128, 128], BF16, tag="trp2")
                            nc.tensor.transpose(
                                tp2[:], Enat[rc][:, m * 128:(m + 1) * 128], identb[:])
                            nc.vector.tensor_copy(
                                ET[m][:, rc * 128:(rc + 1) * 128], tp2[:])

                # Z precompute: Z[(tsub,b), gate @ chunk mc] for t = 8*mc + tsub
                Wst = [wp.tile([128, G4], BF16, name=f"wst{k}", tag=f"wst{k}")
                       for k in range(4)]
                for k in range(4):
                    nc.sync.dma_start(Wst[k][:], wst_d[k])
                with tc.tile_pool(name="pZ", bufs=1, space="PSUM") as pZ:
                    for c in range(4):
                        zps = pZ.tile([128, G4], F32, tag="zps")
                        lhs = [ET[0][:, c * 128:(c + 1) * 128],
                               ET[1][:, c * 128:(c + 1) * 128],
                               Frep[0][:], Frep[1][:]]
                        for j in range(4):
                            sl = slice(j * 512, (j + 1) * 512)
                            for k in range(4):
                                nc.tensor.matmul(zps[:, sl], lhs[k], Wst[k][:, sl],
                                                 start=(k == 0), stop=False)
                            nc.tensor.matmul(zps[:, sl], ones[0:1, :],
                                             ball_s[0:1, sl], start=False, stop=True)
                        if c % 2 == 0:
                            nc.scalar.copy(Z[:, c * G4:(c + 1) * G4], zps[:])
                        else:
                            nc.vector.tensor_copy(Z[:, c * G4:(c + 1) * G4], zps[:])

            # W_fc^T fully resident (bf16, 10MB) + bias
            WFC = [sp.tile([128, VOCAB], BF16, name=f"wfcs{k}", tag=f"wfcs{k}")
                   for k in range(4)]
            for k in range(4):
                nc.sync.dma_start(WFC[k][:], wfc_d[k])
            bfc_s = sp.tile([1, VOCAB], BF16, tag="bfc_s")
            nc.sync.dma_start(bfc_s[:], bfc_d[:])

            # FC work queue: (mc, v0, w) — unit (mc, ...) becomes runnable
            # once LSTM step 8*mc+7 has written its h; interleave up to 3
            # units into each step's PE gap, drain the rest after the loop.
            vchunks = []
            v0 = 0
            while v0 < VOCAB:
                vchunks.append((v0, min(512, VOCAB - v0)))
                v0 += 512
            fc_queue = [(mc, v0, w) for mc in range(4) for (v0, w) in vchunks]
            fc_pos = 0

            # ---------------- LSTM loop + interleaved FC ----------------
            with (
                tc.tile_pool(name="lps", bufs=1, space="PSUM") as lps,
                tc.tile_pool(name="trpp", bufs=2, space="PSUM") as trpp,
                tc.tile_pool(name="fps2", bufs=2, space="PSUM") as fps2,
                tc.tile_pool(name="lsb", bufs=2) as lsb,
                tc.tile_pool(name="fsb2", bufs=4) as fsb2,
            ):
                def fc_unit(mc, v0, w):
                    hsl = slice((8 * mc + 1) * 16, (8 * mc + 9) * 16)
                    ps = fps2.tile([128, 512], F32, tag="fcp")
                    for k in range(4):
                        nc.tensor.matmul(
                            ps[:, :w], hsT[k][:, hsl], WFC[k][:, v0:v0 + w],
                            start=(k == 0), stop=False)
                    nc.tensor.matmul(
                        ps[:, :w], ones[0:1, :], bfc_s[0:1, v0:v0 + w],
                        start=False, stop=True)
                    stg = fsb2.tile([128, 512], F32, tag="fst")
                    if v0 % 1024 == 0:
                        nc.scalar.copy(stg[:, :w], ps[:, :w])
                    else:
                        nc.vector.tensor_copy(stg[:, :w], ps[:, :w])
                    nc.sync.dma_start(
                        out_d[:, 8 * mc:8 * mc + 8, v0:v0 + w]
                        .transpose([1, 0, 2]),
                        stg[:, :w])

                STEP_MS = 0.012
                for t in range(T):
                    t0 = 0.15 + STEP_MS * t
                    # select step-t rows of Z from its 32-aligned row pair:
                    # lhsT = I16 block picking lower/upper 16 of the group
                    zrow = 32 * ((t % 8) // 2)
                    zsel = (sels[zrow:zrow + 32, 0:16] if t % 2 == 0
                            else sels[zrow:zrow + 32, 16:32])
                    zoff = (t // 8) * G4
                    gps = lps.tile([BL, G4], F32, tag="gps")
                    with tc.tile_wait_until(t0):
                        for j in range(4):
                            sl = slice(j * 512, (j + 1) * 512)
                            nc.tensor.matmul(
                                gps[:, sl], zsel,
                                Z[zrow:zrow + 32,
                                  zoff + j * 512:zoff + (j + 1) * 512],
                                start=True, stop=False, tile_position=(zrow, 0))
                            for k in range(4):
                                nc.tensor.matmul(
                                    gps[:, sl], hsT[k][:, t * 16:t * 16 + 16],
                                    U[k][:, sl], start=False, stop=(k == 3))
                    # fill the elementwise-chain PE gap with ready FC units:
                    # these are independent of h_t, so they execute while the
                    # ACT/DVE chain below produces h_t (logical time puts them
                    # after this step's recurrence matmuls on the PE)
                    with tc.tile_wait_until(t0 + 0.5 * STEP_MS):
                        nfill = 0
                        while (fc_pos < len(fc_queue) and nfill < 3
                               and 8 * fc_queue[fc_pos][0] + 7 <= t - 1):
                            fc_unit(*fc_queue[fc_pos])
                            fc_pos += 1
                            nfill += 1
                    sact = lsb.tile([BL, G4], F32, tag="sact")
                    nc.scalar.activation(sact[:, 0:512], gps[:, 0:512],
                                         mybir.ActivationFunctionType.Tanh)
                    nc.scalar.activation(sact[:, 512:2048], gps[:, 512:2048],
                                         mybir.ActivationFunctionType.Sigmoid)
                    ig = lsb.tile([BL, 512], F32, tag="ig")
                    nc.vector.tensor_tensor(ig[:], sact[:, 512:1024],
                                            sact[:, 0:512], mybir.AluOpType.mult)
                    fc = lsb.tile([BL, 512], F32, tag="fc")
                    nc.vector.tensor_tensor(fc[:], sact[:, 1024:1536],
                                            c_sb[:], mybir.AluOpType.mult)
                    nc.vector.tensor_tensor(c_sb[:], ig[:], fc[:],
                                            mybir.AluOpType.add)
                    tch = lsb.tile([BL, 512], F32, tag="tch")
                    nc.scalar.activation(tch[:], c_sb[:],
                                         mybir.ActivationFunctionType.Tanh)
                    h = lsb.tile([BL, 512], F32, tag="h")
                    nc.vector.tensor_tensor(h[:], sact[:, 1536:2048],
                                            tch[:], mybir.AluOpType.mult)
                    tp3 = trpp.tile([128, 4 * BL], F32, tag="htr")
                    for k in range(4):
                        nc.tensor.transpose(
                            tp3[:, k * BL:(k + 1) * BL],
                            h[:, k * 128:(k + 1) * 128], ident[:BL, :BL])
                        nc.vector.tensor_copy(
                            hsT[k][:, (t + 1) * 16:(t + 2) * 16],
                            tp3[:, k * BL:(k + 1) * BL])
                # drain the remaining FC units
                while fc_pos < len(fc_queue):
                    fc_unit(*fc_queue[fc_pos])
                    fc_pos += 1

    nc.compile()
    _CACHE["nc"] = nc
    return nc


def _host_prep(inputs):
    """Build the 8 per-core input maps from full-size inputs."""
    f32 = lambda x: np.ascontiguousarray(np.asarray(x), dtype=np.float32)
    features = f32(inputs["features"])
    captions = np.asarray(inputs["captions"])
    emb = f32(inputs["emb"])
    w_ft = f32(inputs["W_ft"])

    # gate order [g(c), i, f, o]
    gates = ["c", "i", "f", "o"]
    w_stack = np.concatenate([f32(inputs[f"W_{g}"]) for g in gates], axis=1)
    u_all = np.concatenate([f32(inputs[f"U_{g}"]) for g in gates], axis=1)
    b_all = np.concatenate([f32(inputs[f"b_{g}"]) for g in gates])
    w_fc = f32(inputs["W_fc"])
    b_fc = f32(inputs["b_fc"])
    b_ft = f32(inputs["b_ft"])

    import ml_dtypes
    bf16 = ml_dtypes.bfloat16

    sel = np.zeros((NRT * 128, BL), np.float32)
    rows = np.arange(ROWS)
    sel[rows, rows // NREG] = 1.0 / NREG
    sel = sel.reshape(NRT, 128, BL)

    shared = {
        "sel": sel.astype(bf16),
        "emb": emb.astype(bf16),
        "wft": np.ascontiguousarray(w_ft.T).reshape(8, 128, EMBED).astype(bf16),
        "bft": b_ft.reshape(1, EMBED).astype(bf16),
        "wst": w_stack.reshape(4, 128, G4).astype(bf16),
        "ball": b_all.reshape(1, G4).astype(bf16),
        "u": u_all.reshape(4, 128, G4).astype(bf16),
        "wfc": np.ascontiguousarray(w_fc.T).reshape(4, 128, VOCAB).astype(bf16),
        "bfc": b_fc.reshape(1, VOCAB).astype(bf16),
    }
    in_maps = []
    for c in range(NCORES):
        bsl = slice(c * BL, (c + 1) * BL)
        idx = np.ascontiguousarray(
            captions[bsl, :T].astype(np.int64).T).astype(np.int32)  # [T, BL]
        m = dict(shared)
        m["feat"] = features[bsl].reshape(ROWS, FEAT).astype(bf16)
        m["idx"] = idx.reshape(4, 128, 1)
        in_maps.append(m)
    return in_maps


def _enable_ntff_hook():
    """Register the axon NTFF profile hook (missing antenv.axon_hooks shim)."""
    import sys
    import types
    try:
        from antenv.axon_hooks import get_axon_ntff_profile_hook  # noqa: F401
        return
    except ImportError:
        pass
    mod = types.ModuleType("antenv.axon_hooks")
    holder = [None]
    mod.set_axon_ntff_profile_hook = lambda h: holder.__setitem__(0, h)
    mod.get_axon_ntff_profile_hook = lambda: holder[0]
    sys.modules["antenv.axon_hooks"] = mod
    import antenv
    antenv.axon_hooks = mod
    from trn_agent_boot.trn_boot import _ntff_profile_via_ctypes
    mod.set_axon_ntff_profile_hook(
        _ntff_profile_via_ctypes("/opt/axon/libaxon_pjrt.so"))
    from concourse import bass_utils as bu
    bu.upload_artifacts = lambda tmpdir: f"local:{tmpdir}"


def kernel(**inputs) -> np.ndarray:
    nc = _build()
    in_maps = _host_prep(inputs)
    trace = bool(int(os.environ.get("DECODER_KERNEL_TRACE", "0")))
    kw = {}
    if trace:
        _enable_ntff_hook()
        tdir = os.environ.get("DECODER_KERNEL_TRACE_DIR")
        if tdir:
            os.makedirs(tdir, exist_ok=True)
            kw["tmpdir"] = tdir
    res = run_bass_kernel_spmd(nc, in_maps, list(range(NCORES)), trace=trace, **kw)
    _CACHE["exec_time_ns"] = res.exec_time_ns
    _CACHE["results_obj"] = res
    out = np.concatenate([res.results[c]["out"] for c in range(NCORES)], axis=0)
    return out


# revision 33
# speedup vs baseline: 1.1710x; 1.1560x over previous
"""DecoderRNN (image-caption LSTM decoder) Trainium2 kernel.

Data-parallel over batch across 8 NeuronCores (16 batch rows per core).
All LSTM/embedding/fc parameters are replicated; each core computes its
batch shard end-to-end (mean-pool + feature projection, embedding gather,
32-step LSTM, vocab projection) and writes its [16, 32, 10000] output slice.

Self-contained: hardcodes all shapes; host-side numpy only reshapes /
transposes / concatenates weights and computes gather indices.
"""

import os
import numpy as np

import concourse.bass as bass
import concourse.mybir as mybir
import concourse.tile as tile
from concourse import bacc
from concourse.bass_utils import run_bass_kernel_spmd
from concourse.masks import make_identity

F32 = mybir.dt.float32
BF16 = mybir.dt.bfloat16
I32 = mybir.dt.int32

EMBED, HIDDEN, VOCAB, FEAT = 256, 512, 10000, 1024
B, NREG, T = 128, 49, 32  # T = CAPLEN - 1
NCORES = 8
BL = B // NCORES          # 16 batch rows per core
ROWS = BL * NREG          # 784 feature rows per core
NRT = (ROWS + 127) // 128  # 7 feature row tiles
G4 = 4 * HIDDEN           # 2048 gate width, order [g(c), i, f, o]
NT = T * BL               # 512 (t, b) rows per core

_CACHE = {}


def _build():
    if "nc" in _CACHE:
        return _CACHE["nc"]

    nc = bacc.Bacc("TRN2", target_bir_lowering=False, debug=False)

    # ---------------- DRAM I/O ----------------
    feat_d = nc.dram_tensor("feat", [ROWS, FEAT], BF16, kind="ExternalInput")
    sel_d = nc.dram_tensor("sel", [NRT, 128, BL], BF16, kind="ExternalInput")
    idx_d = nc.dram_tensor("idx", [4, 128, 1], I32, kind="ExternalInput")
    emb_d = nc.dram_tensor("emb", [VOCAB, EMBED], BF16, kind="ExternalInput")
    wft_d = nc.dram_tensor("wft", [8, 128, EMBED], BF16, kind="ExternalInput")
    bft_d = nc.dram_tensor("bft", [1, EMBED], BF16, kind="ExternalInput")
    wst_d = nc.dram_tensor("wst", [4, 128, G4], BF16, kind="ExternalInput")
    ball_d = nc.dram_tensor("ball", [1, G4], BF16, kind="ExternalInput")
    u_d = nc.dram_tensor("u", [4, 128, G4], BF16, kind="ExternalInput")
    wfc_d = nc.dram_tensor("wfc", [4, 128, VOCAB], BF16, kind="ExternalInput")
    bfc_d = nc.dram_tensor("bfc", [1, VOCAB], BF16, kind="ExternalInput")
    out_d = nc.dram_tensor("out", [BL, T, VOCAB], F32, kind="ExternalOutput")

    with tile.TileContext(nc) as tc:
        with (
            tc.tile_pool(name="const", bufs=1) as cp,
            tc.tile_pool(name="state", bufs=1) as sp,
        ):
            ident = cp.tile([128, 128], F32, tag="ident")
            make_identity(nc, ident[:])
            identb = cp.tile([128, 128], BF16, tag="identb")
            nc.vector.tensor_copy(identb[:], ident[:])
            ones = cp.tile([1, 128], BF16, tag="ones")
            nc.gpsimd.memset(ones[:], 1.0)
            # 32x32 identity tiled down all partitions: sels[p, q] = (p%32 == q)
            sels = cp.tile([128, 32], BF16, tag="sels")
            for a in range(4):
                nc.vector.tensor_copy(sels[32 * a:32 * (a + 1), :],
                                      ident[0:32, 0:32])

            # persistent state
            U = [sp.tile([128, G4], BF16, name=f"u{k}", tag=f"u{k}")
                 for k in range(4)]
            for k in range(4):
                nc.sync.dma_start(U[k][:], u_d[k])
            # per-step gate pre-acts; 8 steps per 2048-wide chunk, step t at
            # partitions [16*(t%8), 16*(t%8)+16) of chunk t//8
            Z = sp.tile([128, 4 * G4], BF16, tag="Z")
            hsT = [sp.tile([128, 16 * (T + 1)], BF16, name=f"hsT{k}", tag=f"hsT{k}")
                   for k in range(4)]                 # h^T history, slot 0 = h0 = 0
            for k in range(4):
                nc.gpsimd.memset(hsT[k][:], 0.0)
            c_sb = sp.tile([BL, HIDDEN], F32, tag="c_sb")  # LSTM cell state
            nc.gpsimd.memset(c_sb[:], 0.0)
            ET = [sp.tile([128, NT], BF16, name=f"ET{m}", tag=f"ET{m}")
                  for m in range(2)]
            Frep = [sp.tile([128, 128], BF16, name=f"Frep{m}", tag=f"Frep{m}")
                    for m in range(2)]

            # ---------------- prologue ----------------
            with tc.tile_pool(name="wpool", bufs=1) as wp:
                ball_s = wp.tile([1, G4], BF16, tag="ball")
                nc.sync.dma_start(ball_s[:], ball_d[:])
                bft_s = wp.tile([1, EMBED], BF16, tag="bft")
                nc.sync.dma_start(bft_s[:], bft_d[:])
                # feature tiles + selection matmul: fm = sel^T @ feat  -> [BL, FEAT]
                with tc.tile_pool(name="pA", bufs=2, space="PSUM") as pA:
                    fm_ps = pA.tile([BL, FEAT], F32, tag="fm_ps", bufs=1)
                    for kt in range(NRT):
                        ft = wp.tile([128, FEAT], BF16, name=f"ft{kt}", tag=f"ft{kt}")
                        r0 = kt * 128
                        nrows = min(128, ROWS - r0)
                        if nrows < 128:
                            nc.gpsimd.memset(ft[:], 0.0)
                        nc.sync.dma_start(ft[:nrows, :], feat_d[r0:r0 + nrows, :])
                        st = wp.tile([128, BL], BF16, name=f"st{kt}", tag=f"st{kt}")
                        nc.sync.dma_start(st[:], sel_d[kt])
                        for j in range(2):
                            nc.tensor.matmul(
                                fm_ps[:, j * 512:(j + 1) * 512], st[:],
                                ft[:, j * 512:(j + 1) * 512],
                                start=(kt == 0), stop=(kt == NRT - 1))
                    fm_sb = wp.tile([BL, FEAT], BF16, tag="fm_sb")
                    nc.scalar.copy(fm_sb[:], fm_ps[:])

                    # transpose fm -> fmT (8 x [128, BL])
                    fmT = [wp.tile([128, BL], BF16, name=f"fmT{kt}", tag=f"fmT{kt}")
                           for kt in range(8)]
                    for kt in range(8):
                        tp = pA.tile([128, BL], BF16, tag="trp")
                        nc.tensor.transpose(
                            tp[:], fm_sb[:, kt * 128:(kt + 1) * 128],
                            identb[:BL, :BL])
                        nc.vector.tensor_copy(fmT[kt][:], tp[:])

                    # feats_emb^T = wftT-chunks^T @ fmT + b_ft  -> fsb [2][128, BL]
                    for m in range(2):
                        fps = pA.tile([128, BL], F32, tag="fps", bufs=1)
                        for kt in range(8):
                            wt = wp.tile([128, EMBED], BF16, name="wtft",
                                         tag="wtft", bufs=2)
                            nc.sync.dma_start(wt[:], wft_d[kt])
                            nc.tensor.matmul(
                                fps[:], wt[:, m * 128:(m + 1) * 128], fmT[kt][:],
                                start=(kt == 0), stop=False)
                        nc.tensor.matmul(
                            fps[:], bft_s[0:1, m * 128:(m + 1) * 128],
                            ones[0:1, :BL], start=False, stop=True)
                        fsb = wp.tile([128, BL], BF16, name=f"fsb{m}", tag=f"fsb{m}")
                        nc.vector.tensor_copy(fsb[:], fps[:])
                        nc.vector.tensor_copy(
                            Frep[m][:].rearrange("p (s b) -> p s b", s=8),
                            fsb[:].unsqueeze(1).to_broadcast([128, 8, BL]))

                    # embedding gather -> E_nat [4][128, EMBED], rows (t, b)
                    Enat = [wp.tile([128, EMBED], BF16, name=f"en{rc}", tag=f"en{rc}")
                            for rc in range(4)]
                    for rc in range(4):
                        it = wp.tile([128, 1], I32, name=f"it{rc}", tag=f"it{rc}")
                        nc.sync.dma_start(it[:], idx_d[rc])
                        nc.gpsimd.indirect_dma_start(
                            out=Enat[rc][:], out_offset=None,
                            in_=emb_d[:],
                            in_offset=bass.IndirectOffsetOnAxis(ap=it[:, 0:1], axis=0))
                    # transpose E_nat -> ET [2][128, NT]
                    for rc in range(4):
                        for m in range(2):
                            tp2 = pA.tile([128, 128], BF16, tag="trp2")
                            nc.tensor.transpose(
                                tp2[:], Enat[rc][:, m * 128:(m + 1) * 128], identb[:])
                            nc.vector.tensor_copy(
                                ET[m][:, rc * 128:(rc + 1) * 128], tp2[:])

                # Z precompute: Z[(tsub,b), gate @ chunk mc] for t = 8*mc + tsub
                Wst = [wp.tile([128, G4], BF16, name=f"wst{k}", tag=f"wst{k}")
                       for k in range(4)]
                for k in range(4):
                    nc.sync.dma_start(Wst[k][:], wst_d[k])
                with tc.tile_pool(name="pZ", bufs=1, space="PSUM") as pZ:
                    for c in range(4):
                        zps = pZ.tile([128, G4], F32, tag="zps")
                        lhs = [ET[0][:, c * 128:(c + 1) * 128],
                               ET[1][:, c * 128:(c + 1) * 128],
                               Frep[0][:], Frep[1][:]]
                        for j in range(4):
                            sl = slice(j * 512, (j + 1) * 512)
                            for k in range(4):
                                nc.tensor.matmul(zps[:, sl], lhs[k], Wst[k][:, sl],
                                                 start=(k == 0), stop=False)
                            nc.tensor.matmul(zps[:, sl], ones[0:1, :],
                                             ball_s[0:1, sl], start=False, stop=True)
                        if c % 2 == 0:
                            nc.scalar.copy(Z[:, c * G4:(c + 1) * G4], zps[:])
                        else:
                            nc.vector.tensor_copy(Z[:, c * G4:(c + 1) * G4], zps[:])

            # W_fc^T fully resident (bf16, 10MB) + bias
            WFC = [sp.tile([128, VOCAB], BF16, name=f"wfcs{k}", tag=f"wfcs{k}")
                   for k in range(4)]
            for k in range(4):
                nc.sync.dma_start(WFC[k][:], wfc_d[k])
            bfc_s = sp.tile([1, VOCAB], BF16, tag="bfc_s")
            nc.sync.dma_start(bfc_s[:], bfc_d[:])

            # FC work queue: (mc, v0, w) — unit (mc, ...) becomes runnable
            # once LSTM step 8*mc+7 has written its h; interleave up to 3
            # units into each step's PE gap, drain the rest after the loop.
            vchunks = []
            v0 = 0
            while v0 < VOCAB:
                vchunks.append((v0, min(512, VOCAB - v0)))
                v0 += 512
            fc_queue = [(mc, v0, w) for mc in range(4) for (v0, w) in vchunks]
            fc_pos = 0

            # ---------------- LSTM loop + interleaved FC ----------------
            with (
                tc.tile_pool(name="lps", bufs=1, space="PSUM") as lps,
                tc.tile_pool(name="trpp", bufs=2, space="PSUM") as trpp,
                tc.tile_pool(name="fps2", bufs=2, space="PSUM") as fps2,
                tc.tile_pool(name="lsb", bufs=2) as lsb,
                tc.tile_pool(name="fsb2", bufs=4) as fsb2,
            ):
                def fc_unit(mc, v0, w):
                    hsl = slice((8 * mc + 1) * 16, (8 * mc + 9) * 16)
                    ps = fps2.tile([128, 512], F32, tag="fcp")
                    for k in range(4):
                        nc.tensor.matmul(
                            ps[:, :w], hsT[k][:, hsl], WFC[k][:, v0:v0 + w],
                            start=(k == 0), stop=False)
                    nc.tensor.matmul(
                        ps[:, :w], ones[0:1, :], bfc_s[0:1, v0:v0 + w],
                        start=False, stop=True)
                    stg = fsb2.tile([128, 512], F32, tag="fst")
                    if v0 % 1024 == 0:
                        nc.scalar.copy(stg[:, :w], ps[:, :w])
                    else:
                        nc.vector.tensor_copy(stg[:, :w], ps[:, :w])
                    nc.sync.dma_start(
                        out_d[:, 8 * mc:8 * mc + 8, v0:v0 + w]
                        .transpose([1, 0, 2]),
                        stg[:, :w])

                STEP_MS = 0.012
                for t in range(T):
                    t0 = 0.15 + STEP_MS * t
                    # select step-t rows of Z from its 32-aligned row pair:
                    # lhsT = I16 block picking lower/upper 16 of the group
                    zrow = 32 * ((t % 8) // 2)
                    zsel = (sels[zrow:zrow + 32, 0:16] if t % 2 == 0
                            else sels[zrow:zrow + 32, 16:32])
                    zoff = (t // 8) * G4
                    gps = [lps.tile([BL, 512], F32, tag=f"gps{j}", name=f"gps{j}")
                           for j in range(4)]
                    with tc.tile_wait_until(t0):
                        for j in range(4):
                            sl = slice(j * 512, (j + 1) * 512)
                            nc.tensor.matmul(
                                gps[j][:], zsel,
                                Z[zrow:zrow + 32,
                                  zoff + j * 512:zoff + (j + 1) * 512],
                                start=True, stop=False, tile_position=(zrow, 0))
                            for k in range(4):
                                nc.tensor.matmul(
                                    gps[j][:], hsT[k][:, t * 16:t * 16 + 16],
                                    U[k][:, sl], start=False, stop=(k == 3))
                    # fill the elementwise-chain PE gap with ready FC units:
                    # these are independent of h_t, so they execute while the
                    # ACT/DVE chain below produces h_t (logical time puts them
                    # after this step's recurrence matmuls on the PE)
                    with tc.tile_wait_until(t0 + 0.5 * STEP_MS):
                        nfill = 0
                        while (fc_pos < len(fc_queue) and nfill < 4
                               and 8 * fc_queue[fc_pos][0] + 7 <= t - 1):
                            fc_unit(*fc_queue[fc_pos])
                            fc_pos += 1
                            nfill += 1
                    sact = lsb.tile([BL, G4], F32, tag="sact")
                    nc.scalar.activation(sact[:, 0:512], gps[0][:],
                                         mybir.ActivationFunctionType.Tanh)
                    for j in range(1, 4):
                        nc.scalar.activation(
                            sact[:, j * 512:(j + 1) * 512], gps[j][:],
                            mybir.ActivationFunctionType.Sigmoid)
                    ig = lsb.tile([BL, 512], F32, tag="ig")
                    nc.vector.tensor_tensor(ig[:], sact[:, 512:1024],
                                            sact[:, 0:512], mybir.AluOpType.mult)
                    fc = lsb.tile([BL, 512], F32, tag="fc")
                    nc.vector.tensor_tensor(fc[:], sact[:, 1024:1536],
                                            c_sb[:], mybir.AluOpType.mult)
                    nc.vector.tensor_tensor(c_sb[:], ig[:], fc[:],
                                            mybir.AluOpType.add)
                    tch = lsb.tile([BL, 512], F32, tag="tch")
                    nc.scalar.activation(tch[:], c_sb[:],
                                         mybir.ActivationFunctionType.Tanh)
                    h = lsb.tile([BL, 512], F32, tag="h")
                    nc.vector.tensor_tensor(h[:], sact[:, 1536:2048],
                                            tch[:], mybir.AluOpType.mult)
                    tp3 = trpp.tile([128, 4 * BL], F32, tag="htr")
                    for k in range(4):
                        nc.tensor.transpose(
                            tp3[:, k * BL:(k + 1) * BL],
                            h[:, k * 128:(k + 1) * 128], ident[:BL, :BL])
                        nc.vector.tensor_copy(
                            hsT[k][:, (t + 1) * 16:(t + 2) * 16],
                            tp3[:, k * BL:(k + 1) * BL])
                # drain the remaining FC units
                while fc_pos < len(fc_queue):
                    fc_unit(*fc_queue[fc_pos])
                    fc_pos += 1

    nc.compile()
    _CACHE["nc"] = nc
    return nc


def _host_prep(inputs):
    """Build the 8 per-core input maps from full-size inputs."""
    f32 = lambda x: np.ascontiguousarray(np.asarray(x), dtype=np.float32)
    features = f32(inputs["features"])
    captions = np.asarray(inputs["captions"])
    emb = f32(inputs["emb"])
    w_ft = f32(inputs["W_ft"])

    # gate order [g(c), i, f, o]
    gates = ["c", "i", "f", "o"]
    w_stack = np.concatenate([f32(inputs[f"W_{g}"]) for g in gates], axis=1)
    u_all = np.concatenate([f32(inputs[f"U_{g}"]) for g in gates], axis=1)
    b_all = np.concatenate([f32(inputs[f"b_{g}"]) for g in gates])
    w_fc = f32(inputs["W_fc"])
    b_fc = f32(inputs["b_fc"])
    b_ft = f32(inputs["b_ft"])

    import ml_dtypes
    bf16 = ml_dtypes.bfloat16

    sel = np.zeros((NRT * 128, BL), np.float32)
    rows = np.arange(ROWS)
    sel[rows, rows // NREG] = 1.0 / NREG
    sel = sel.reshape(NRT, 128, BL)

    shared = {
        "sel": sel.astype(bf16),
        "emb": emb.astype(bf16),
        "wft": np.ascontiguousarray(w_ft.T).reshape(8, 128, EMBED).astype(bf16),
        "bft": b_ft.reshape(1, EMBED).astype(bf16),
        "wst": w_stack.reshape(4, 128, G4).astype(bf16),
        "ball": b_all.reshape(1, G4).astype(bf16),
        "u": u_all.reshape(4, 128, G4).astype(bf16),
        "wfc": np.ascontiguousarray(w_fc.T).reshape(4, 128, VOCAB).astype(bf16),
        "bfc": b_fc.reshape(1, VOCAB).astype(bf16),
    }
    in_maps = []
    for c in range(NCORES):
        bsl = slice(c * BL, (c + 1) * BL)
        idx = np.ascontiguousarray(
            captions[bsl, :T].astype(np.int64).T).astype(np.int32)  # [T, BL]
        m = dict(shared)
        m["feat"] = features[bsl].reshape(ROWS, FEAT).astype(bf16)
        m["idx"] = idx.reshape(4, 128, 1)
        in_maps.append(m)
    return in_maps


def _enable_ntff_hook():
    """Register the axon NTFF profile hook (missing antenv.axon_hooks shim)."""
    import sys
    import types
    try:
        from antenv.axon_hooks import get_axon_ntff_profile_hook  # noqa: F401
        return
    except ImportError:
        pass
    mod = types.ModuleType("antenv.axon_hooks")
    holder = [None]
    mod.set_axon_ntff_profile_hook = lambda h: holder.__setitem__(0, h)
    mod.get_axon_ntff_profile_hook = lambda: holder[0]
    sys.modules["antenv.axon_hooks"] = mod
    import antenv
    antenv.axon_hooks = mod
    from trn_agent_boot.trn_boot import _ntff_profile_via_ctypes
    mod.set_axon_ntff_profile_hook(
        _ntff_profile_via_ctypes("/opt/axon/libaxon_pjrt.so"))
    from concourse import bass_utils as bu
    bu.upload_artifacts = lambda tmpdir: f"local:{tmpdir}"


def kernel(**inputs) -> np.ndarray:
    nc = _build()
    in_maps = _host_prep(inputs)
    trace = bool(int(os.environ.get("DECODER_KERNEL_TRACE", "0")))
    kw = {}
    if trace:
        _enable_ntff_hook()
        tdir = os.environ.get("DECODER_KERNEL_TRACE_DIR")
        if tdir:
            os.makedirs(tdir, exist_ok=True)
            kw["tmpdir"] = tdir
    res = run_bass_kernel_spmd(nc, in_maps, list(range(NCORES)), trace=trace, **kw)
    _CACHE["exec_time_ns"] = res.exec_time_ns
    _CACHE["results_obj"] = res
    out = np.concatenate([res.results[c]["out"] for c in range(NCORES)], axis=0)
    return out
